# revision 8
# baseline (speedup 1.0000x reference)
"""DifferentiableQuantizer Trainium2 kernel.

Math (from the reference):
    discrete_bits = snap(bit_assignment, {2,4,8})        # [B, G]
    group_bits    = floor(mean_B(discrete_bits))         # [G]
    qmax_g        = 2**group_bits - 1                    # [G]
    qmax_d        = qmax_g[group_indices]                # [D]
    s  = max(scale, 1e-8); xs = x / s + zp
    out = (clip(round(xs), 0, qmax_d) - zp) * s          # [B, S, D]

The table math is tiny ([8,16] and [1024]) and runs on host. The heavy part
is a pure elementwise pass over x [8, 4096, 1024] f32 (128 MiB in + 128 MiB
out), which is memory-bound. Sharding: split the D=1024 channels into 8
slices of 128 (= SBUF partition count); each core processes all B*S rows for
its 128 channels with the per-channel constants living in [128, 1]
per-partition scalars. Host transposes x to channel-major so every DMA is
contiguous along the free axis.

Device program per tile [128, F]:
    round:  t = (t + 1.5*2^23) - 1.5*2^23     (one DVE tensor_scalar, RNE)
    clip:   t = max(min(t, qmax), 0)          (one DVE tensor_scalar)
plus optional affine pre/post ops when scale/zero_point are non-trivial.
"""

import numpy as np

import concourse.bass as bass
import concourse.mybir as mybir
import concourse.tile as tile
from concourse import bacc
from concourse.bass_utils import run_bass_kernel_spmd

N_CORES = 8
B, S, D, G = 8, 4096, 1024, 16
ROWS = B * S              # 32768 elements per channel
P = D // N_CORES          # 128 channels per core == SBUF partitions
F = 2048                  # free-dim tile size (8 KiB per partition line)
N_TILES = ROWS // F
BUFS = 8

MAGIC = 12582912.0        # 1.5 * 2**23: fp32 add/sub rounds to nearest-even
EPS = 1e-8

# Stash of the last run's results so test.py can read exec_time_ns.
LAST_RESULTS = None


def _build(trivial_affine: bool) -> bass.Bass:
    # Bacc (not raw Bass): its compile() runs generate_event_semaphores,
    # which splits multi-sem waits — TRN2 allows only one wait per
    # instruction and walrus rejects the BIR otherwise.
    nc = bacc.Bacc("TRN2", debug=False, num_devices=N_CORES)
    op = mybir.AluOpType
    f32 = mybir.dt.float32

    x = nc.dram_tensor("x", [P, ROWS], f32, kind="ExternalInput").ap()
    qmax = nc.dram_tensor("qmax", [P, 1], f32, kind="ExternalInput").ap()
    if not trivial_affine:
        a_in = nc.dram_tensor("a", [P, 1], f32, kind="ExternalInput").ap()
        b_in = nc.dram_tensor("b", [P, 1], f32, kind="ExternalInput").ap()
        s_in = nc.dram_tensor("s", [P, 1], f32, kind="ExternalInput").ap()
        d_in = nc.dram_tensor("d", [P, 1], f32, kind="ExternalInput").ap()
    out = nc.dram_tensor("out", [P, ROWS], f32, kind="ExternalOutput").ap()

    with tile.TileContext(nc) as tc:
        with (
            tc.tile_pool(name="const", bufs=1) as cpool,
            tc.tile_pool(name="work", bufs=BUFS) as pool,
        ):
            # Constants are DMA'd into a staging tile, then copied on DVE so
            # that consumers only ever depend on the DVE semaphore — the
            # walrus TensorScalarPtr lowering rejects instructions that need
            # more than one sync wait (DVE sem + DMAHW sem).
            def load_const(src, tag):
                raw = cpool.tile([P, 1], f32, tag=tag + "_raw")
                dst = cpool.tile([P, 1], f32, tag=tag)
                # gpsimd (SWDGE) keeps the tiny const transfer off the two
                # HWDGE rings that stream the bulk tiles.
                nc.gpsimd.dma_start(raw[:], src)
                nc.vector.tensor_copy(dst[:], raw[:])
                return dst

            qv = load_const(qmax, "qv")
            if not trivial_affine:
                av = load_const(a_in, "av")
                bv = load_const(b_in, "bv")
                sv = load_const(s_in, "sv")
                dv = load_const(d_in, "dv")

            for i in range(N_TILES):
                t = pool.tile([P, F], f32)
                sl = slice(i * F, (i + 1) * F)
                nc.sync.dma_start(t[:], x[:, sl])
                if not trivial_affine:
                    # xs = x * (1/s) + zp
                    nc.vector.tensor_scalar(
                        t[:], t[:], av[:], bv[:], op0=op.mult, op1=op.add
                    )
                # round to nearest even
                nc.vector.tensor_scalar(
                    t[:], t[:], MAGIC, MAGIC, op0=op.add, op1=op.subtract
                )
                # clip to [0, qmax]
                nc.vector.tensor_scalar(
                    t[:], t[:], qv[:], 0.0, op0=op.min, op1=op.max
                )
                if not trivial_affine:
                    # (q - zp) * s == q * s + (-zp * s)
                    nc.vector.tensor_scalar(
                        t[:], t[:], sv[:], dv[:], op0=op.mult, op1=op.add
                    )
                # Stores on the second HWDGE ring (scalar/ACT) so load and
                # store issue don't share one FIFO.
                nc.scalar.dma_start(out[:, sl], t[:])
    nc.compile()
    return nc


def kernel(x, scale, zero_point, bit_assignment, group_indices):
    global LAST_RESULTS
    x = np.asarray(x, dtype=np.float32)
    scale = np.asarray(scale, dtype=np.float32).reshape(-1)          # [D]
    zero_point = np.asarray(zero_point, dtype=np.float32).reshape(-1)
    bit_assignment = np.asarray(bit_assignment, dtype=np.float32)    # [B, G]
    group_indices = np.asarray(group_indices)                        # [D] int32

    # --- host: per-channel qmax table -----------------------------------
    levels = np.array([2.0, 4.0, 8.0], dtype=np.float32)
    dist = np.abs(bit_assignment[..., None] - levels)                # [B, G, 3]
    discrete = levels[np.argmin(dist, axis=-1)]                      # [B, G]
    group_bits = np.floor(discrete.mean(axis=0, dtype=np.float32))   # [G]
    qmax_g = (np.float32(2.0) ** group_bits - np.float32(1.0)).astype(np.float32)
    qmax_d = qmax_g[group_indices].astype(np.float32)                # [D]

    s_eff = np.maximum(scale, np.float32(EPS))
    trivial = bool(np.all(s_eff == 1.0) and np.all(zero_point == 0.0))

    # --- host: shard to channel-major per-core blocks -------------------
    xt = np.ascontiguousarray(x.reshape(ROWS, D).T)                  # [D, ROWS]

    in_maps = []
    for c in range(N_CORES):
        ch = slice(c * P, (c + 1) * P)
        m = {
            "x": xt[ch],
            "qmax": np.ascontiguousarray(qmax_d[ch]).reshape(P, 1),
        }
        if not trivial:
            m["a"] = (1.0 / s_eff[ch]).astype(np.float32).reshape(P, 1)
            m["b"] = zero_point[ch].astype(np.float32).reshape(P, 1)
            m["s"] = s_eff[ch].astype(np.float32).reshape(P, 1)
            m["d"] = (-zero_point[ch] * s_eff[ch]).astype(np.float32).reshape(P, 1)
        in_maps.append(m)

    nc = _build(trivial)
    LAST_RESULTS = run_bass_kernel_spmd(nc, in_maps, core_ids=list(range(N_CORES)))

    out_t = np.concatenate(
        [LAST_RESULTS.results[c]["out"] for c in range(N_CORES)], axis=0
    )                                                                # [D, ROWS]
    return np.ascontiguousarray(out_t.T).reshape(B, S, D)


# revision 10
# speedup vs baseline: 1.0209x; 1.0209x over previous
"""DifferentiableQuantizer Trainium2 kernel.

Math (from the reference):
    discrete_bits = snap(bit_assignment, {2,4,8})        # [B, G]
    group_bits    = floor(mean_B(discrete_bits))         # [G]
    qmax_g        = 2**group_bits - 1                    # [G]
    qmax_d        = qmax_g[group_indices]                # [D]
    s  = max(scale, 1e-8); xs = x / s + zp
    out = (clip(round(xs), 0, qmax_d) - zp) * s          # [B, S, D]

The table math is tiny ([8,16] and [1024]) and runs on host. The heavy part
is a pure elementwise pass over x [8, 4096, 1024] f32 (128 MiB in + 128 MiB
out), which is memory-bound. Sharding: split the D=1024 channels into 8
slices of 128 (= SBUF partition count); each core processes all B*S rows for
its 128 channels with the per-channel constants living in [128, 1]
per-partition scalars. Host transposes x to channel-major so every DMA is
contiguous along the free axis.

Device program per tile [128, F]:
    round:  t = (t + 1.5*2^23) - 1.5*2^23     (one DVE tensor_scalar, RNE)
    clip:   t = max(min(t, qmax), 0)          (one DVE tensor_scalar)
plus optional affine pre/post ops when scale/zero_point are non-trivial.
"""

import numpy as np

import concourse.bass as bass
import concourse.mybir as mybir
import concourse.tile as tile
from concourse import bacc
from concourse.bass_utils import run_bass_kernel_spmd

N_CORES = 8
B, S, D, G = 8, 4096, 1024, 16
ROWS = B * S              # 32768 elements per channel
P = D // N_CORES          # 128 channels per core == SBUF partitions
F = 2048                  # free-dim tile size (8 KiB per partition line)
N_TILES = ROWS // F
BUFS = 12

MAGIC = 12582912.0        # 1.5 * 2**23: fp32 add/sub rounds to nearest-even
EPS = 1e-8

# Stash of the last run's results so test.py can read exec_time_ns.
LAST_RESULTS = None


def _build(trivial_affine: bool) -> bass.Bass:
    # Bacc (not raw Bass): its compile() runs generate_event_semaphores,
    # which splits multi-sem waits — TRN2 allows only one wait per
    # instruction and walrus rejects the BIR otherwise.
    nc = bacc.Bacc("TRN2", debug=False, num_devices=N_CORES)
    op = mybir.AluOpType
    f32 = mybir.dt.float32

    x = nc.dram_tensor("x", [P, ROWS], f32, kind="ExternalInput").ap()
    qmax = nc.dram_tensor("qmax", [P, 1], f32, kind="ExternalInput").ap()
    if not trivial_affine:
        a_in = nc.dram_tensor("a", [P, 1], f32, kind="ExternalInput").ap()
        b_in = nc.dram_tensor("b", [P, 1], f32, kind="ExternalInput").ap()
        s_in = nc.dram_tensor("s", [P, 1], f32, kind="ExternalInput").ap()
        d_in = nc.dram_tensor("d", [P, 1], f32, kind="ExternalInput").ap()
    out = nc.dram_tensor("out", [P, ROWS], f32, kind="ExternalOutput").ap()

    with tile.TileContext(nc) as tc:
        with (
            tc.tile_pool(name="const", bufs=1) as cpool,
            tc.tile_pool(name="work", bufs=BUFS) as pool,
        ):
            # Constants are DMA'd into a staging tile, then copied on DVE so
            # that consumers only ever depend on the DVE semaphore — the
            # walrus TensorScalarPtr lowering rejects instructions that need
            # more than one sync wait (DVE sem + DMAHW sem).
            def load_const(src, tag):
                raw = cpool.tile([P, 1], f32, tag=tag + "_raw")
                dst = cpool.tile([P, 1], f32, tag=tag)
                nc.sync.dma_start(raw[:], src)
                nc.vector.tensor_copy(dst[:], raw[:])
                return dst

            qv = load_const(qmax, "qv")
            if not trivial_affine:
                av = load_const(a_in, "av")
                bv = load_const(b_in, "bv")
                sv = load_const(s_in, "sv")
                dv = load_const(d_in, "dv")

            for i in range(N_TILES):
                t = pool.tile([P, F], f32)
                sl = slice(i * F, (i + 1) * F)
                nc.sync.dma_start(t[:], x[:, sl])
                if not trivial_affine:
                    # xs = x * (1/s) + zp
                    nc.vector.tensor_scalar(
                        t[:], t[:], av[:], bv[:], op0=op.mult, op1=op.add
                    )
                # round to nearest even
                nc.vector.tensor_scalar(
                    t[:], t[:], MAGIC, MAGIC, op0=op.add, op1=op.subtract
                )
                # clip to [0, qmax]
                nc.vector.tensor_scalar(
                    t[:], t[:], qv[:], 0.0, op0=op.min, op1=op.max
                )
                if not trivial_affine:
                    # (q - zp) * s == q * s + (-zp * s)
                    nc.vector.tensor_scalar(
                        t[:], t[:], sv[:], dv[:], op0=op.mult, op1=op.add
                    )
                # Stores on the second HWDGE ring (scalar/ACT) so load and
                # store issue don't share one FIFO.
                nc.scalar.dma_start(out[:, sl], t[:])
    nc.compile()
    return nc


def kernel(x, scale, zero_point, bit_assignment, group_indices):
    global LAST_RESULTS
    x = np.asarray(x, dtype=np.float32)
    scale = np.asarray(scale, dtype=np.float32).reshape(-1)          # [D]
    zero_point = np.asarray(zero_point, dtype=np.float32).reshape(-1)
    bit_assignment = np.asarray(bit_assignment, dtype=np.float32)    # [B, G]
    group_indices = np.asarray(group_indices)                        # [D] int32

    # --- host: per-channel qmax table -----------------------------------
    levels = np.array([2.0, 4.0, 8.0], dtype=np.float32)
    dist = np.abs(bit_assignment[..., None] - levels)                # [B, G, 3]
    discrete = levels[np.argmin(dist, axis=-1)]                      # [B, G]
    group_bits = np.floor(discrete.mean(axis=0, dtype=np.float32))   # [G]
    qmax_g = (np.float32(2.0) ** group_bits - np.float32(1.0)).astype(np.float32)
    qmax_d = qmax_g[group_indices].astype(np.float32)                # [D]

    s_eff = np.maximum(scale, np.float32(EPS))
    trivial = bool(np.all(s_eff == 1.0) and np.all(zero_point == 0.0))

    # --- host: shard to channel-major per-core blocks -------------------
    xt = np.ascontiguousarray(x.reshape(ROWS, D).T)                  # [D, ROWS]

    in_maps = []
    for c in range(N_CORES):
        ch = slice(c * P, (c + 1) * P)
        m = {
            "x": xt[ch],
            "qmax": np.ascontiguousarray(qmax_d[ch]).reshape(P, 1),
        }
        if not trivial:
            m["a"] = (1.0 / s_eff[ch]).astype(np.float32).reshape(P, 1)
            m["b"] = zero_point[ch].astype(np.float32).reshape(P, 1)
            m["s"] = s_eff[ch].astype(np.float32).reshape(P, 1)
            m["d"] = (-zero_point[ch] * s_eff[ch]).astype(np.float32).reshape(P, 1)
        in_maps.append(m)

    nc = _build(trivial)
    LAST_RESULTS = run_bass_kernel_spmd(nc, in_maps, core_ids=list(range(N_CORES)))

    out_t = np.concatenate(
        [LAST_RESULTS.results[c]["out"] for c in range(N_CORES)], axis=0
    )                                                                # [D, ROWS]
    return np.ascontiguousarray(out_t.T).reshape(B, S, D)
